# revision 19
# baseline (speedup 1.0000x reference)
"""Trainium2 Bass kernel for nn_ContrastiveNetWithGATAndTransformer.

Encoder MLP -> GATConv (2 heads, segment softmax) -> 2-layer transformer
(4 heads, post-LN, relu FFN) over the full 4096-node sequence.

Distribution over 8 NeuronCores (SPMD, one program, zero core-dependent
constants — all per-core variation is carried by per-core input data):
- Encoder + GAT node features (hh, a_src, a_dst): replicated compute.
- GAT edge aggregation: dst-sharded (512 dst / core; edges sorted by dst on
  the host, hh rows fetched via gpsimd dma_gather, segment softmax +
  aggregation via one-hot scatter matmuls on the PE).
- In-kernel AllGather of y^T between stages.
- Transformer: q-sharded (512 query rows / core); K/V computed redundantly
  from the gathered full y^T.
"""
import os
import sys

for _p in ("/opt/trn_rl_repo", "/root/.axon_site/_ro/trn_rl_repo"):
    if os.path.isdir(_p) and _p not in sys.path:
        sys.path.append(_p)

import numpy as np
import ml_dtypes

import concourse.bass as bass
import concourse.mybir as mybir
import concourse.tile as tile
from concourse import bacc
from concourse.bass_utils import run_bass_kernel_spmd

BF = ml_dtypes.bfloat16
N, F_IN, H, C, D, NH, DH, L, FF = 4096, 256, 2, 64, 128, 4, 32, 2, 2048
NCORES = 8
NT = 32                # global dst tiles of 128 nodes
TPC = NT // NCORES     # dst tiles per core (4)
TW = 256               # gather table row width in bf16 (= 512B)
EPS = 1e-5
ISQ = 1.0 / np.sqrt(np.float32(DH))
DEBUG = bool(int(os.environ.get("KBUILD_DEBUG", "0")))

f32 = mybir.dt.float32
bf16 = mybir.dt.bfloat16
i16 = mybir.dt.int16

AF = mybir.ActivationFunctionType
OP = mybir.AluOpType


def _wrap16(stream):
    """[..., m] index stream -> [..., 128, m//16] int16 (16-wrap, x8 groups)."""
    m = stream.shape[-1]
    w = stream.reshape(*stream.shape[:-1], m // 16, 16)
    w = np.swapaxes(w, -1, -2)  # [..., 16, m//16]
    out = np.zeros((*stream.shape[:-1], 128, m // 16), np.int16)
    for g in range(8):
        out[..., 16 * g : 16 * (g + 1), :] = w
    return out


# ----------------------------------------------------------------- host prep
def _host_prep(inputs):
    inp = {k: np.asarray(v) for k, v in inputs.items()}
    src = inp["edge_index"][0].astype(np.int64)
    dst = inp["edge_index"][1].astype(np.int64)
    order = np.argsort(dst, kind="stable")
    src, dst = src[order], dst[order]
    gtile = dst >> 7
    counts = np.bincount(gtile, minlength=NT)
    maxb = int(np.ceil(counts.max() / 128))
    maxb = ((maxb + 7) // 8) * 8          # blocks per dst tile, multiple of 8
    ncalls = maxb // 8                     # 1024-index dma_gather calls/tile
    npad = maxb * 128

    src_pad = np.zeros((NT, npad), np.int16)
    dstrel = np.full((NT, npad), 255.0, np.float32)
    off = np.concatenate([[0], np.cumsum(counts)])
    for g in range(NT):
        s, e = off[g], off[g + 1]
        src_pad[g, : e - s] = src[s:e]
        dstrel[g, : e - s] = (dst[s:e] & 127).astype(np.float32)

    gidx = _wrap16(src_pad.reshape(NT, ncalls, 1024))       # [NT,nc,128,64]
    dstrel_b = dstrel.reshape(NT, maxb, 128).astype(np.float32)
    # per-tile dst node ids (for the a_dst mini-gather)
    dt_ids = (np.arange(NT)[:, None] * 128
              + np.arange(128)[None, :]).astype(np.int16)   # [NT,128]
    dtidx = _wrap16(dt_ids)                                 # [NT,128,8]

    def b(x):
        return np.ascontiguousarray(np.asarray(x)).astype(BF)

    def f(x):
        return np.ascontiguousarray(np.asarray(x)).astype(np.float32)

    iota = np.broadcast_to(np.arange(128, dtype=np.float32), (128, 128))
    ident = np.eye(128, dtype=np.float32)

    asrcW = np.zeros((128, 2), np.float32)   # lhsT for a_src: [hc, head]
    adstW = np.zeros((128, 2), np.float32)
    for h in range(H):
        asrcW[h * C : (h + 1) * C, h] = inp["gat_asrc"][h]
        adstW[h * C : (h + 1) * C, h] = inp["gat_adst"][h]

    def bcast(v):
        return np.broadcast_to(np.asarray(v, np.float32), (128, 128)).copy()

    w2s = inp["t_w2"].reshape(L, 16, 128, 128).transpose(0, 2, 1, 3)
    b1c = inp["t_b1"].reshape(L, 16, 128).transpose(0, 2, 1)

    shared = {
        "xT": f(inp["x"].T),
        "ew1": f(inp["enc_w1"]),
        "eb1": f(inp["enc_b1"][:, None]),
        "ew2": b(inp["enc_w2"]),
        "eb2": f(inp["enc_b2"][:, None]),
        "gatw": b(inp["gat_w"]),
        "asrcW": b(asrcW),
        "adstW": b(adstW),
        "gbias_b": f(bcast(inp["gat_bias"])),
        "iota_b": b(iota),
        "id_f32": f(ident),
        "id_bf": b(ident),
        "wq": b(inp["t_wq"]), "wk": b(inp["t_wk"]), "wv": b(inp["t_wv"]),
        "wo": b(inp["t_wo"]),
        "bq": f(inp["t_bq"][:, :, None]),
        "bk": f(inp["t_bk"][:, :, None]),
        "bv_b": f(np.stack([bcast(inp["t_bv"][l]) for l in range(L)])),
        "bo_b": f(np.stack([bcast(inp["t_bo"][l]) for l in range(L)])),
        "w1": b(inp["t_w1"]),
        "b1c": f(b1c),
        "w2s": b(w2s),
        "b2_b": f(np.stack([bcast(inp["t_b2"][l]) for l in range(L)])),
        "ln1g_b": f(np.stack([bcast(inp["t_ln1_g"][l]) for l in range(L)])),
        "ln1b_b": f(np.stack([bcast(inp["t_ln1_b"][l]) for l in range(L)])),
        "ln2g_b": f(np.stack([bcast(inp["t_ln2_g"][l]) for l in range(L)])),
        "ln2b_b": f(np.stack([bcast(inp["t_ln2_b"][l]) for l in range(L)])),
    }
    in_maps = []
    for c in range(NCORES):
        m = dict(shared)
        m["gidx"] = np.ascontiguousarray(gidx[c * TPC : (c + 1) * TPC])
        m["dstrel"] = np.ascontiguousarray(dstrel_b[c * TPC : (c + 1) * TPC])
        m["dtidx"] = np.ascontiguousarray(dtidx[c * TPC : (c + 1) * TPC])
        in_maps.append(m)
    return in_maps, maxb, ncalls


# ----------------------------------------------------------------- kernel IR
def _build(maxb, ncalls):
    nc = bacc.Bacc("TRN2", target_bir_lowering=False, debug=False,
                   num_devices=NCORES)
    P = {}

    def din(name, shape, dt):
        P[name] = nc.declare_dram_parameter(name, list(shape), dt,
                                            isOutput=False).ap()

    din("xT", (F_IN, N), f32)
    din("ew1", (F_IN, D), f32); din("eb1", (D, 1), f32)
    din("ew2", (D, D), bf16);   din("eb2", (D, 1), f32)
    din("gatw", (D, D), bf16)
    din("asrcW", (D, 2), bf16); din("adstW", (D, 2), bf16)
    din("gbias_b", (D, D), f32)
    din("iota_b", (128, 128), bf16)
    din("id_f32", (128, 128), f32); din("id_bf", (128, 128), bf16)
    din("gidx", (TPC, ncalls, 128, 64), i16)
    din("dstrel", (TPC, maxb, 128), f32)
    din("dtidx", (TPC, 128, 8), i16)
    din("wq", (L, D, D), bf16); din("wk", (L, D, D), bf16)
    din("wv", (L, D, D), bf16); din("wo", (L, D, D), bf16)
    din("bq", (L, D, 1), f32);  din("bk", (L, D, 1), f32)
    din("bv_b", (L, D, D), f32); din("bo_b", (L, D, D), f32)
    din("w1", (L, D, FF), bf16); din("b1c", (L, 128, 16), f32)
    din("w2s", (L, 128, 16, 128), bf16); din("b2_b", (L, D, D), f32)
    for nm in ("ln1g_b", "ln1b_b", "ln2g_b", "ln2b_b"):
        din(nm, (L, D, D), f32)

    y_out = nc.declare_dram_parameter("y_out", [512, D], f32, isOutput=True).ap()
    dbg = {}
    if DEBUG:
        for nm, shape in (("dbg_hT", (D, N)), ("dbg_hhT", (D, N)),
                          ("dbg_aT", (2, 2 * N)), ("dbg_y", (512, D)),
                          ("dbg_y1", (512, D))):
            dbg[nm] = nc.declare_dram_parameter(nm, list(shape), f32,
                                                isOutput=True).ap()

    tableT = nc.dram_tensor("tableT", [N, TW], bf16).ap()
    ag_in = [nc.dram_tensor(f"ag_in{i}", [D, 512], bf16).ap() for i in range(2)]
    ag_out = [nc.dram_tensor(f"ag_out{i}", [NCORES, D, 512], bf16,
                             addr_space="Shared").ap() for i in range(2)]

    with tile.TileContext(nc) as tc:
        from contextlib import ExitStack
        stack = ExitStack()
        consts = stack.enter_context(tc.tile_pool(name="consts", bufs=1))
        persist = stack.enter_context(tc.tile_pool(name="persist", bufs=1))

        def load_const(name, shape, dt, src_ap):
            t = consts.tile(list(shape), dt, tag=name)
            nc.sync.dma_start(out=t[:], in_=src_ap)
            return t

        iota_b = load_const("iota_b", (128, 128), bf16, P["iota_b"][:, :])
        id_f = load_const("id_f32", (128, 128), f32, P["id_f32"][:, :])
        id_b = load_const("id_bf", (128, 128), bf16, P["id_bf"][:, :])
        gbias = load_const("gbias_b", (D, D), f32, P["gbias_b"][:, :])
        eps_t = consts.tile([128, 1], f32, tag="eps")
        nc.vector.memset(eps_t[:], EPS)

        yT_full = persist.tile([D, N], bf16, tag="yT_full")
        y_nat = persist.tile([128, TPC, D], f32, tag="y_nat")
        y1_nat = persist.tile([128, TPC, D], f32, tag="y1_nat")
        y2_nat = persist.tile([128, TPC, D], f32, tag="y2_nat")
        yT_loc = persist.tile([D, 512], bf16, tag="yT_loc")

        # ---------------- stage 1: encoder + tables (replicated) ----------
        with (
            tc.tile_pool(name="enc", bufs=1) as enc,
            tc.tile_pool(name="encw", bufs=2) as encw,
            tc.tile_pool(name="pse", bufs=2, space="PSUM") as pse,
        ):
            xT0 = enc.tile([128, N], f32, tag="xT0")
            xT1 = enc.tile([128, N], f32, tag="xT1")
            nc.sync.dma_start(out=xT0[:], in_=P["xT"][0:128, :])
            nc.sync.dma_start(out=xT1[:], in_=P["xT"][128:256, :])
            ew1a = consts.tile([128, D], f32, tag="ew1a")
            ew1b = consts.tile([128, D], f32, tag="ew1b")
            nc.sync.dma_start(out=ew1a[:], in_=P["ew1"][0:128, :])
            nc.sync.dma_start(out=ew1b[:], in_=P["ew1"][128:256, :])
            ew2 = load_const("ew2", (D, D), bf16, P["ew2"][:, :])
            eb1 = load_const("eb1", (D, 1), f32, P["eb1"][:, :])
            eb2 = load_const("eb2", (D, 1), f32, P["eb2"][:, :])
            gatw = load_const("gatw", (D, D), bf16, P["gatw"][:, :])
            asrcW = load_const("asrcW", (D, 2), bf16, P["asrcW"][:, :])
            adstW = load_const("adstW", (D, 2), bf16, P["adstW"][:, :])

            h1T = enc.tile([D, N], bf16, tag="h1T")
            for nb in range(8):
                sl = slice(512 * nb, 512 * (nb + 1))
                ps = pse.tile([128, 512], f32, tag="pse")
                nc.tensor.matmul(out=ps[:], lhsT=ew1a[:], rhs=xT0[:, sl],
                                 start=True, stop=False)
                nc.tensor.matmul(out=ps[:], lhsT=ew1b[:], rhs=xT1[:, sl],
                                 start=False, stop=True)
                nc.scalar.activation(out=h1T[:, sl], in_=ps[:], func=AF.Relu,
                                     bias=eb1[:], scale=1.0)
            hT = enc.tile([D, N], bf16, tag="hT")
            hhT = enc.tile([D, N], bf16, tag="hhT")
            for nb in range(8):
                sl = slice(512 * nb, 512 * (nb + 1))
                ps = pse.tile([128, 512], f32, tag="pse")
                nc.tensor.matmul(out=ps[:], lhsT=ew2[:], rhs=h1T[:, sl],
                                 start=True, stop=True)
                nc.scalar.activation(out=hT[:, sl], in_=ps[:], func=AF.Identity,
                                     bias=eb2[:], scale=1.0)
            for nb in range(8):
                sl = slice(512 * nb, 512 * (nb + 1))
                ps = pse.tile([128, 512], f32, tag="pse")
                nc.tensor.matmul(out=ps[:], lhsT=gatw[:], rhs=hT[:, sl],
                                 start=True, stop=True)
                nc.vector.tensor_copy(out=hhT[:, sl], in_=ps[:])

            asT = enc.tile([2, N], f32, tag="asT")
            adT = enc.tile([2, N], f32, tag="adT")
            for nb in range(8):
                sl = slice(512 * nb, 512 * (nb + 1))
                ps = pse.tile([2, 512], f32, tag="psa")
                nc.tensor.matmul(out=ps[:], lhsT=asrcW[:], rhs=hhT[:, sl],
                                 start=True, stop=True)
                nc.vector.tensor_copy(out=asT[:, sl], in_=ps[:])
                ps2 = pse.tile([2, 512], f32, tag="psa")
                nc.tensor.matmul(out=ps2[:], lhsT=adstW[:], rhs=hhT[:, sl],
                                 start=True, stop=True)
                nc.vector.tensor_copy(out=adT[:, sl], in_=ps2[:])

            if DEBUG:
                dhT = enc.tile([D, N], f32, tag="dhT")
                nc.gpsimd.tensor_copy(out=dhT[:], in_=hT[:])
                nc.sync.dma_start(out=dbg["dbg_hT"][:, :], in_=dhT[:])
                dhhT = enc.tile([D, N], f32, tag="dhhT")
                nc.gpsimd.tensor_copy(out=dhhT[:], in_=hhT[:])
                nc.sync.dma_start(out=dbg["dbg_hhT"][:, :], in_=dhhT[:])
                daT = enc.tile([2, 2 * N], f32, tag="daT")
                nc.gpsimd.tensor_copy(out=daT[:, 0:N], in_=asT[:])
                nc.gpsimd.tensor_copy(out=daT[:, N : 2 * N], in_=adT[:])
                nc.sync.dma_start(out=dbg["dbg_aT"][:, :], in_=daT[:])

            # gather table rows: [hh(n) x128 | asrc(n) x2 | adst(n) x2 | pad]
            for nt in range(32):
                sl = slice(128 * nt, 128 * (nt + 1))
                pst = pse.tile([128, 128], bf16, tag="pst")
                nc.tensor.transpose(out=pst[:], in_=hhT[:, sl], identity=id_b[:])
                row = encw.tile([128, TW], bf16, tag="row")
                nc.vector.tensor_copy(out=row[:, 0:128], in_=pst[:])
                psu = pse.tile([128, 128], f32, tag="psu")
                nc.tensor.transpose(out=psu[:], in_=asT[:, sl],
                                    identity=id_f[0:2, :])
                nc.vector.tensor_copy(out=row[:, 128:130], in_=psu[:, 0:2])
                psu2 = pse.tile([128, 128], f32, tag="psu")
                nc.tensor.transpose(out=psu2[:], in_=adT[:, sl],
                                    identity=id_f[0:2, :])
                nc.vector.tensor_copy(out=row[:, 130:132], in_=psu2[:, 0:2])
                nc.sync.dma_start(out=tableT[sl, :], in_=row[:])

        # ---------------- stage 2: GAT edge aggregation -------------------
        with (
            tc.tile_pool(name="gg", bufs=2) as gg,          # gathered blocks
            tc.tile_pool(name="gix", bufs=2) as gix,
            tc.tile_pool(name="gsm", bufs=3) as gsm,        # small per-block
            tc.tile_pool(name="gtl", bufs=2) as gtl,        # per-tile
            tc.tile_pool(name="psg", bufs=2, space="PSUM") as psg,
            tc.tile_pool(name="psx", bufs=2, space="PSUM") as psx,
        ):
            for t in range(TPC):
                # a_dst for this tile's 128 dst nodes, via mini dma_gather
                dti = gix.tile([128, 8], i16, tag="dti")
                nc.sync.dma_start(out=dti[:], in_=P["dtidx"][t, :, :])
                drow = gtl.tile([128, 1, TW], bf16, tag="drow")
                nc.gpsimd.dma_gather(out_ap=drow[:, :, :],
                                     in_ap=tableT[:, :], idxs_ap=dti[:, :],
                                     num_idxs=128, num_idxs_reg=128,
                                     elem_size=TW)
                # AdstB[h][p, f] = a_dst[tile_node f, head h]
                adstB = gtl.tile([128, H, 128], bf16, tag="adstB")
                for h in range(H):
                    pst = psx.tile([128, 128], bf16, tag="pstgb")
                    nc.tensor.transpose(out=pst[0:1, :],
                                        in_=drow[:, 0, 130 + h : 131 + h],
                                        identity=id_b[:])
                    adrow = gtl.tile([1, 128], bf16, tag="adrow")
                    nc.vector.tensor_copy(out=adrow[:], in_=pst[0:1, :])
                    nc.gpsimd.partition_broadcast(
                        out_ap=adstB[:, h, :], in_ap=adrow[:],
                        channels=128)

                agg = psg.tile([128, 130], f32, tag="agg")
                for call in range(ncalls):
                    git = gix.tile([128, 64], i16, tag="git")
                    nc.sync.dma_start(out=git[:], in_=P["gidx"][t, call, :, :])
                    G = gg.tile([128, 8, TW], bf16, tag="G")
                    nc.gpsimd.dma_gather(out_ap=G[:, :, :],
                                         in_ap=tableT[:, :], idxs_ap=git[:, :],
                                         num_idxs=1024, num_idxs_reg=1024,
                                         elem_size=TW)
                    for bb in range(8):
                        blk = call * 8 + bb
                        drel = gsm.tile([128, 1], f32, tag="drel")
                        nc.sync.dma_start(out=drel[:],
                                          in_=P["dstrel"][t, blk, :, None])
                        ST = gsm.tile([128, 128], bf16, tag="ST")
                        nc.vector.tensor_scalar(out=ST[:], in0=iota_b[:],
                                                scalar1=drel[:], scalar2=None,
                                                op0=OP.is_equal)
                        # a_dst per edge: rowsum(ST * AdstB[h])
                        ade = gsm.tile([128, 2], f32, tag="ade")
                        tmp = gsm.tile([128, 128], bf16, tag="tmpm")
                        for h in range(H):
                            nc.vector.tensor_tensor(out=tmp[:], in0=ST[:],
                                                    in1=adstB[:, h, :],
                                                    op=OP.mult)
                            nc.vector.reduce_sum(out=ade[:, h : h + 1],
                                                 in_=tmp[:],
                                                 axis=mybir.AxisListType.X)
                        # phi = a_src(gathered) + a_dst ; ee = exp(lrelu(phi))
                        phi = gsm.tile([128, 2], f32, tag="phi")
                        nc.vector.tensor_tensor(out=phi[:],
                                                in0=G[:, bb, 128:130],
                                                in1=ade[:], op=OP.add)
                        lr = gsm.tile([128, 2], f32, tag="lr")
                        nc.vector.tensor_scalar(out=lr[:], in0=phi[:],
                                                scalar1=0.2, scalar2=None,
                                                op0=OP.mult)
                        nc.vector.tensor_tensor(out=lr[:], in0=lr[:],
                                                in1=phi[:], op=OP.max)
                        ee = gsm.tile([128, 2], f32, tag="ee")
                        nc.scalar.activation(out=ee[:], in_=lr[:], func=AF.Exp)
                        # vals = [ee0*hh0 | ee1*hh1 | ee]
                        vals = gsm.tile([128, 130], bf16, tag="vals")
                        for h in range(H):
                            nc.vector.tensor_scalar(
                                out=vals[:, 64 * h : 64 * (h + 1)],
                                in0=G[:, bb, 64 * h : 64 * (h + 1)],
                                scalar1=ee[:, h : h + 1], scalar2=None,
                                op0=OP.mult)
                        nc.vector.tensor_copy(out=vals[:, 128:130], in_=ee[:])
                        nc.tensor.matmul(out=agg[:], lhsT=ST[:], rhs=vals[:],
                                         start=(blk == 0),
                                         stop=(blk == maxb - 1))
                # epilogue: y = agg/denom + bias ; yT
                rec = gsm.tile([128, 2], f32, tag="rec")
                nc.vector.reciprocal(out=rec[:], in_=agg[:, 128:130])
                yt = gtl.tile([128, D], f32, tag="yt")
                for h in range(H):
                    nc.vector.tensor_scalar(out=yt[:, 64 * h : 64 * (h + 1)],
                                            in0=agg[:, 64 * h : 64 * (h + 1)],
                                            scalar1=rec[:, h : h + 1],
                                            scalar2=None, op0=OP.mult)
                nc.vector.tensor_tensor(out=y_nat[:, t, :], in0=yt[:],
                                        in1=gbias[:], op=OP.add)
                pyt = psx.tile([128, 128], f32, tag="pstg")
                nc.tensor.transpose(out=pyt[:], in_=y_nat[:, t, :],
                                    identity=id_f[:])
                nc.vector.tensor_copy(out=yT_loc[:, 128 * t : 128 * (t + 1)],
                                      in_=pyt[:])
            if DEBUG:
                for t in range(TPC):
                    nc.sync.dma_start(out=dbg["dbg_y"][128 * t : 128 * (t + 1), :],
                                      in_=y_nat[:, t, :])

        # ---------------- transformer ------------------------------------
        def allgather(idx):
            nc.sync.dma_start(out=ag_in[idx][:, :], in_=yT_loc[:])
            nc.gpsimd.collective_compute(
                "AllGather", OP.bypass,
                ins=[ag_in[idx]], outs=[ag_out[idx]],
                replica_groups=[list(range(NCORES))])
            nc.sync.dma_start(
                out=yT_full[:].rearrange("d (c n) -> d c n", c=NCORES),
                in_=ag_out[idx].rearrange("c d n -> d c n"))

        allgather(0)

        def cload(nm, shape, dt, src):
            t = consts.tile(list(shape), dt, tag=nm, name=nm)
            nc.sync.dma_start(out=t[:], in_=src)
            return t

        wq_s, wk_s, wv_s, bq_s, bk_s = [], [], [], [], []
        wo_h = [[None] * L for _ in range(NH)]
        w1_s, w2_s, b1c_s = [], [], []
        bcasts = {nm: [] for nm in ("bv_b", "bo_b", "b2_b", "ln1g_b",
                                    "ln1b_b", "ln2g_b", "ln2b_b")}
        for l in range(L):
            wq_s.append(cload(f"wq{l}", (D, D), bf16, P["wq"][l, :, :]))
            wk_s.append(cload(f"wk{l}", (D, D), bf16, P["wk"][l, :, :]))
            wv_s.append(cload(f"wv{l}", (D, D), bf16, P["wv"][l, :, :]))
            for h in range(NH):
                wo_h[h][l] = cload(f"wo{l}_{h}", (DH, D), bf16,
                                   P["wo"][l, DH * h : DH * (h + 1), :])
            bq_s.append(cload(f"bq{l}", (D, 1), f32, P["bq"][l, :, :]))
            bk_s.append(cload(f"bk{l}", (D, 1), f32, P["bk"][l, :, :]))
            for nm in bcasts:
                bcasts[nm].append(cload(f"{nm}{l}", (D, D), f32,
                                        P[nm][l, :, :]))
            w1_s.append(cload(f"w1{l}", (D, FF), bf16, P["w1"][l, :, :]))
            w2_s.append(cload(f"w2{l}", (128, 16, 128), bf16,
                              P["w2s"][l, :, :, :]))
            b1c_s.append(cload(f"b1c{l}", (128, 16), f32, P["b1c"][l, :, :]))

        def layer_norm(dst_ap, src_sb, g_b, b_b, lnp):
            st = lnp.tile([128, 6], f32, tag="lnst")
            nc.vector.bn_stats(out=st[:], in_=src_sb)
            mv = lnp.tile([128, 2], f32, tag="lnmv")
            nc.vector.bn_aggr(out=mv[:], in_=st[:])
            sd = lnp.tile([128, 1], f32, tag="lnsd")
            nc.scalar.activation(out=sd[:], in_=mv[:, 1:2], func=AF.Sqrt,
                                 bias=eps_t[:], scale=1.0)
            nc.vector.reciprocal(out=sd[:], in_=sd[:])
            xc = lnp.tile([128, D], f32, tag="lnxc")
            nc.vector.tensor_scalar(out=xc[:], in0=src_sb,
                                    scalar1=mv[:, 0:1], scalar2=sd[:],
                                    op0=OP.subtract, op1=OP.mult)
            nc.vector.tensor_tensor(out=xc[:], in0=xc[:], in1=g_b, op=OP.mult)
            nc.vector.tensor_tensor(out=dst_ap, in0=xc[:], in1=b_b, op=OP.add)

        for l in range(L):
            yin_nat = y_nat if l == 0 else y2_nat
            yin_T = yT_loc
            with (
                tc.tile_pool(name="tf", bufs=2) as tf,
                tc.tile_pool(name="lnp", bufs=2) as lnp,
                tc.tile_pool(name="es", bufs=3) as es,
                tc.tile_pool(name="psS", bufs=2, space="PSUM") as psS,
                tc.tile_pool(name="psV", bufs=2, space="PSUM") as psV,
                tc.tile_pool(name="psM", bufs=2, space="PSUM") as psM,
            ):
                KT_a = tf.tile([64, N], bf16, tag="KT_a")
                KT_b = tf.tile([64, N], bf16, tag="KT_b")
                for nb in range(8):
                    sl = slice(512 * nb, 512 * (nb + 1))
                    ps = psM.tile([128, 512], f32, tag="psm")
                    nc.tensor.matmul(out=ps[:], lhsT=wk_s[l][:], rhs=yT_full[:, sl],
                                     start=True, stop=True)
                    nc.vector.tensor_scalar(out=KT_a[:, sl], in0=ps[0:64, :],
                                            scalar1=bk_s[l][0:64, :],
                                            scalar2=None, op0=OP.add)
                    nc.vector.tensor_scalar(out=KT_b[:, sl], in0=ps[64:128, :],
                                            scalar1=bk_s[l][64:128, :],
                                            scalar2=None, op0=OP.add)
                vaug = tf.tile([128, 32, NH * 33], bf16, tag="vaug")
                nc.vector.memset(
                    vaug[:].rearrange("p n (h t) -> p n h t", t=33)[:, :, :, 32:33],
                    1.0)
                for nb in range(32):
                    sl = slice(128 * nb, 128 * (nb + 1))
                    ps = psM.tile([128, 512], f32, tag="psm")
                    nc.tensor.matmul(out=ps[:, 0:128], lhsT=yT_full[:, sl],
                                     rhs=wv_s[l][:], start=True, stop=True)
                    nc.vector.tensor_tensor(
                        out=vaug[:, nb, :].rearrange("p (h t) -> p h t",
                                                     t=33)[:, :, 0:32],
                        in0=ps[:, 0:128].rearrange("p (h c) -> p h c", c=32),
                        in1=bcasts["bv_b"][l][:].rearrange("p (h c) -> p h c",
                                                        c=32),
                        op=OP.add)
                QT_a = tf.tile([64, 512], bf16, tag="QT_a")
                QT_b = tf.tile([64, 512], bf16, tag="QT_b")
                ps = psM.tile([128, 512], f32, tag="psm")
                nc.tensor.matmul(out=ps[:], lhsT=wq_s[l][:], rhs=yin_T[:],
                                 start=True, stop=True)
                nc.scalar.activation(out=QT_a[:], in_=ps[0:64, :],
                                     func=AF.Identity, bias=bq_s[l][0:64, :],
                                     scale=1.0)
                nc.scalar.activation(out=QT_b[:], in_=ps[64:128, :],
                                     func=AF.Identity, bias=bq_s[l][64:128, :],
                                     scale=1.0)

                pv01 = psV.tile([97, 512], f32, tag="pv")
                pv23 = psV.tile([97, 512], f32, tag="pv")
                for h in range(NH):
                    pv = pv01 if h < 2 else pv23
                    prow = 64 * (h % 2)
                    for j in range(16):
                        sc = psS.tile([128, 1024], f32, tag="sc")
                        KTh = KT_a if h < 2 else KT_b
                        QTh = QT_a if h < 2 else QT_b
                        hr = DH * (h % 2)
                        for u in range(2):
                            kc = 2 * j + u
                            nc.tensor.matmul(
                                out=sc[:, 512 * u : 512 * (u + 1)],
                                lhsT=KTh[hr : hr + DH,
                                         128 * kc : 128 * (kc + 1)],
                                rhs=QTh[hr : hr + DH, :],
                                start=True, stop=True)
                        ex = es.tile([128, 1024], bf16, tag="ex")
                        nc.scalar.activation(out=ex[:], in_=sc[:], func=AF.Exp,
                                             scale=float(ISQ))
                        for u in range(2):
                            kc = 2 * j + u
                            nc.tensor.matmul(
                                out=pv[prow : prow + 33, :],
                                lhsT=vaug[:, kc, 33 * h : 33 * (h + 1)],
                                rhs=ex[:, 512 * u : 512 * (u + 1)],
                                start=(kc == 0), stop=(kc == 31))
                an_h = []
                for h in range(NH):
                    pv = pv01 if h < 2 else pv23
                    prow = 64 * (h % 2)
                    ah = tf.tile([33, 512], f32, tag=f"ah{h}", name=f"ah{h}")
                    nc.vector.tensor_copy(out=ah[:],
                                          in_=pv[prow : prow + 33, :])
                    rc = lnp.tile([1, 512], f32, tag="rc")
                    nc.vector.reciprocal(out=rc[:], in_=ah[32:33, :])
                    rbh = lnp.tile([33, 512], f32, tag=f"rb{h}", name=f"rb{h}")
                    nc.gpsimd.partition_broadcast(out_ap=rbh[:], in_ap=rc[:],
                                                  channels=33)
                    anh = tf.tile([33, 512], bf16, tag=f"an{h}", name=f"an{h}")
                    nc.vector.tensor_tensor(out=anh[:], in0=ah[:],
                                            in1=rbh[:], op=OP.mult)
                    an_h.append(anh)
                psP = psM.tile([128, 512], f32, tag="psm")
                for h in range(NH):
                    nc.tensor.matmul(out=psP[:], lhsT=wo_h[h][l][:],
                                     rhs=an_h[h][0:32, :],
                                     start=(h == 0), stop=(h == NH - 1))
                outT = tf.tile([D, 512], bf16, tag="outT")
                nc.vector.tensor_copy(out=outT[:], in_=psP[:])

                # natural: residual + bias + LN1 ; build y1T
                y1T = tf.tile([D, 512], bf16, tag="y1T")
                for q in range(4):
                    pn = psV.tile([128, 512], bf16, tag="pv", name="pn")
                    nc.tensor.transpose(out=pn[:, 0:128],
                                        in_=outT[:, 128 * q : 128 * (q + 1)],
                                        identity=id_b[:])
                    res = lnp.tile([128, D], f32, tag="res")
                    nc.vector.tensor_tensor(out=res[:], in0=pn[:, 0:128],
                                            in1=bcasts["bo_b"][l][:], op=OP.add)
                    nc.vector.tensor_tensor(out=res[:], in0=res[:],
                                            in1=yin_nat[:, q, :], op=OP.add)
                    layer_norm(y1_nat[:, q, :], res[:], bcasts["ln1g_b"][l][:],
                               bcasts["ln1b_b"][l][:], lnp)
                    pt = psM.tile([128, 512], f32, tag="psm")
                    nc.tensor.transpose(out=pt[:, 0:128], in_=y1_nat[:, q, :],
                                        identity=id_f[:])
                    nc.vector.tensor_copy(out=y1T[:, 128 * q : 128 * (q + 1)],
                                          in_=pt[:, 0:128])

                # FFN
                f1T = tf.tile([128, 16, 512], bf16, tag="f1T")
                for fc in range(16):
                    ps = psM.tile([128, 512], f32, tag="psm")
                    nc.tensor.matmul(out=ps[:],
                                     lhsT=w1_s[l][:, 128 * fc : 128 * (fc + 1)],
                                     rhs=y1T[:], start=True, stop=True)
                    nc.vector.tensor_scalar(out=f1T[:, fc, :], in0=ps[:],
                                            scalar1=b1c_s[l][:, fc : fc + 1],
                                            scalar2=0.0, op0=OP.add, op1=OP.max)
                psG = psM.tile([128, 512], f32, tag="psm")
                for fc in range(16):
                    nc.tensor.matmul(out=psG[:], lhsT=w2_s[l][:, fc, :],
                                     rhs=f1T[:, fc, :], start=(fc == 0),
                                     stop=(fc == 15))
                f2T = tf.tile([D, 512], bf16, tag="f2T")
                nc.vector.tensor_copy(out=f2T[:], in_=psG[:])
                for q in range(4):
                    pn = psV.tile([128, 512], bf16, tag="pv", name="pn")
                    nc.tensor.transpose(out=pn[:, 0:128],
                                        in_=f2T[:, 128 * q : 128 * (q + 1)],
                                        identity=id_b[:])
                    res = lnp.tile([128, D], f32, tag="res")
                    nc.vector.tensor_tensor(out=res[:], in0=pn[:, 0:128],
                                            in1=bcasts["b2_b"][l][:], op=OP.add)
                    nc.vector.tensor_tensor(out=res[:], in0=res[:],
                                            in1=y1_nat[:, q, :], op=OP.add)
                    layer_norm(y2_nat[:, q, :], res[:], bcasts["ln2g_b"][l][:],
                               bcasts["ln2b_b"][l][:], lnp)
                    pt = psM.tile([128, 512], f32, tag="psm")
                    nc.tensor.transpose(out=pt[:, 0:128], in_=y2_nat[:, q, :],
                                        identity=id_f[:])
                    nc.vector.tensor_copy(out=yT_loc[:, 128 * q : 128 * (q + 1)],
                                          in_=pt[:, 0:128])
            if l == 0:
                allgather(1)
                if DEBUG:
                    for t in range(TPC):
                        nc.sync.dma_start(
                            out=dbg["dbg_y1"][128 * t : 128 * (t + 1), :],
                            in_=y2_nat[:, t, :])

        for q in range(4):
            nc.sync.dma_start(out=y_out[128 * q : 128 * (q + 1), :],
                              in_=y2_nat[:, q, :])
        stack.close()
    nc.compile()
    return nc


_CACHE = {}


def kernel(**inputs) -> np.ndarray:
    in_maps, maxb, ncalls = _host_prep(inputs)
    key = (maxb, ncalls)
    if key not in _CACHE:
        _CACHE[key] = _build(maxb, ncalls)
    nc = _CACHE[key]
    res = run_bass_kernel_spmd(nc, in_maps, core_ids=list(range(NCORES)))
    out = np.concatenate([res.results[c]["y_out"] for c in range(NCORES)], 0)
    if DEBUG:
        kernel.last_results = res
    return out.astype(np.float32)


# revision 20
# speedup vs baseline: 1.1340x; 1.1340x over previous
"""Trainium2 Bass kernel for nn_ContrastiveNetWithGATAndTransformer.

Encoder MLP -> GATConv (2 heads, segment softmax) -> 2-layer transformer
(4 heads, post-LN, relu FFN) over the full 4096-node sequence.

Distribution over 8 NeuronCores (SPMD, one program, zero core-dependent
constants — all per-core variation is carried by per-core input data):
- Encoder + GAT node features (hh, a_src, a_dst): replicated compute.
- GAT edge aggregation: dst-sharded (512 dst / core; edges sorted by dst on
  the host, hh rows fetched via gpsimd dma_gather, segment softmax +
  aggregation via one-hot scatter matmuls on the PE).
- In-kernel AllGather of y^T between stages.
- Transformer: q-sharded (512 query rows / core); K/V computed redundantly
  from the gathered full y^T.
"""
import os
import sys

for _p in ("/opt/trn_rl_repo", "/root/.axon_site/_ro/trn_rl_repo"):
    if os.path.isdir(_p) and _p not in sys.path:
        sys.path.append(_p)

import numpy as np
import ml_dtypes

import concourse.bass as bass
import concourse.mybir as mybir
import concourse.tile as tile
from concourse import bacc
from concourse.bass_utils import run_bass_kernel_spmd

BF = ml_dtypes.bfloat16
N, F_IN, H, C, D, NH, DH, L, FF = 4096, 256, 2, 64, 128, 4, 32, 2, 2048
NCORES = 8
NT = 32                # global dst tiles of 128 nodes
TPC = NT // NCORES     # dst tiles per core (4)
TW = 256               # gather table row width in bf16 (= 512B)
EPS = 1e-5
ISQ = 1.0 / np.sqrt(np.float32(DH))
DEBUG = bool(int(os.environ.get("KBUILD_DEBUG", "0")))

f32 = mybir.dt.float32
bf16 = mybir.dt.bfloat16
i16 = mybir.dt.int16

AF = mybir.ActivationFunctionType
OP = mybir.AluOpType


def _wrap16(stream):
    """[..., m] index stream -> [..., 128, m//16] int16 (16-wrap, x8 groups)."""
    m = stream.shape[-1]
    w = stream.reshape(*stream.shape[:-1], m // 16, 16)
    w = np.swapaxes(w, -1, -2)  # [..., 16, m//16]
    out = np.zeros((*stream.shape[:-1], 128, m // 16), np.int16)
    for g in range(8):
        out[..., 16 * g : 16 * (g + 1), :] = w
    return out


# ----------------------------------------------------------------- host prep
def _host_prep(inputs):
    inp = {k: np.asarray(v) for k, v in inputs.items()}
    src = inp["edge_index"][0].astype(np.int64)
    dst = inp["edge_index"][1].astype(np.int64)
    order = np.argsort(dst, kind="stable")
    src, dst = src[order], dst[order]
    gtile = dst >> 7
    counts = np.bincount(gtile, minlength=NT)
    maxb = int(np.ceil(counts.max() / 128))
    maxb = ((maxb + 7) // 8) * 8          # blocks per dst tile, multiple of 8
    ncalls = maxb // 8                     # 1024-index dma_gather calls/tile
    npad = maxb * 128

    src_pad = np.zeros((NT, npad), np.int16)
    dstrel = np.full((NT, npad), 255.0, np.float32)
    off = np.concatenate([[0], np.cumsum(counts)])
    for g in range(NT):
        s, e = off[g], off[g + 1]
        src_pad[g, : e - s] = src[s:e]
        dstrel[g, : e - s] = (dst[s:e] & 127).astype(np.float32)

    gidx = _wrap16(src_pad.reshape(NT, ncalls, 1024))       # [NT,nc,128,64]
    dstrel_b = dstrel.reshape(NT, maxb, 128).astype(np.float32)
    # per-tile dst node ids (for the a_dst mini-gather)
    dt_ids = (np.arange(NT)[:, None] * 128
              + np.arange(128)[None, :]).astype(np.int16)   # [NT,128]
    dtidx = _wrap16(dt_ids)                                 # [NT,128,8]

    def b(x):
        return np.ascontiguousarray(np.asarray(x)).astype(BF)

    def f(x):
        return np.ascontiguousarray(np.asarray(x)).astype(np.float32)

    iota = np.broadcast_to(np.arange(128, dtype=np.float32), (128, 128))
    ident = np.eye(128, dtype=np.float32)

    asrcW = np.zeros((128, 2), np.float32)   # lhsT for a_src: [hc, head]
    adstW = np.zeros((128, 2), np.float32)
    for h in range(H):
        asrcW[h * C : (h + 1) * C, h] = inp["gat_asrc"][h]
        adstW[h * C : (h + 1) * C, h] = inp["gat_adst"][h]

    def bcast(v):
        return np.broadcast_to(np.asarray(v, np.float32), (128, 128)).copy()

    w2s = inp["t_w2"].reshape(L, 16, 128, 128).transpose(0, 2, 1, 3)
    b1c = inp["t_b1"].reshape(L, 16, 128).transpose(0, 2, 1)

    shared = {
        "xT": f(inp["x"].T),
        "ew1": f(inp["enc_w1"]),
        "eb1": f(inp["enc_b1"][:, None]),
        "ew2": b(inp["enc_w2"]),
        "eb2": f(inp["enc_b2"][:, None]),
        "gatw": b(inp["gat_w"]),
        "asrcW": b(asrcW),
        "adstW": b(adstW),
        "gbias_b": f(bcast(inp["gat_bias"])),
        "iota_b": b(iota),
        "id_f32": f(ident),
        "id_bf": b(ident),
        "wq": b(inp["t_wq"]), "wk": b(inp["t_wk"]), "wv": b(inp["t_wv"]),
        "wo": b(inp["t_wo"]),
        "bq": f(inp["t_bq"][:, :, None]),
        "bk": f(inp["t_bk"][:, :, None]),
        "bv_b": f(np.stack([bcast(inp["t_bv"][l]) for l in range(L)])),
        "bo_b": f(np.stack([bcast(inp["t_bo"][l]) for l in range(L)])),
        "w1": b(inp["t_w1"]),
        "b1c": f(b1c),
        "w2s": b(w2s),
        "b2_b": f(np.stack([bcast(inp["t_b2"][l]) for l in range(L)])),
        "ln1g_b": f(np.stack([bcast(inp["t_ln1_g"][l]) for l in range(L)])),
        "ln1b_b": f(np.stack([bcast(inp["t_ln1_b"][l]) for l in range(L)])),
        "ln2g_b": f(np.stack([bcast(inp["t_ln2_g"][l]) for l in range(L)])),
        "ln2b_b": f(np.stack([bcast(inp["t_ln2_b"][l]) for l in range(L)])),
    }
    in_maps = []
    for c in range(NCORES):
        m = dict(shared)
        m["gidx"] = np.ascontiguousarray(gidx[c * TPC : (c + 1) * TPC])
        m["dstrel"] = np.ascontiguousarray(dstrel_b[c * TPC : (c + 1) * TPC])
        m["dtidx"] = np.ascontiguousarray(dtidx[c * TPC : (c + 1) * TPC])
        in_maps.append(m)
    return in_maps, maxb, ncalls


# ----------------------------------------------------------------- kernel IR
def _build(maxb, ncalls):
    nc = bacc.Bacc("TRN2", target_bir_lowering=False, debug=False,
                   num_devices=NCORES)
    P = {}

    def din(name, shape, dt):
        P[name] = nc.declare_dram_parameter(name, list(shape), dt,
                                            isOutput=False).ap()

    din("xT", (F_IN, N), f32)
    din("ew1", (F_IN, D), f32); din("eb1", (D, 1), f32)
    din("ew2", (D, D), bf16);   din("eb2", (D, 1), f32)
    din("gatw", (D, D), bf16)
    din("asrcW", (D, 2), bf16); din("adstW", (D, 2), bf16)
    din("gbias_b", (D, D), f32)
    din("iota_b", (128, 128), bf16)
    din("id_f32", (128, 128), f32); din("id_bf", (128, 128), bf16)
    din("gidx", (TPC, ncalls, 128, 64), i16)
    din("dstrel", (TPC, maxb, 128), f32)
    din("dtidx", (TPC, 128, 8), i16)
    din("wq", (L, D, D), bf16); din("wk", (L, D, D), bf16)
    din("wv", (L, D, D), bf16); din("wo", (L, D, D), bf16)
    din("bq", (L, D, 1), f32);  din("bk", (L, D, 1), f32)
    din("bv_b", (L, D, D), f32); din("bo_b", (L, D, D), f32)
    din("w1", (L, D, FF), bf16); din("b1c", (L, 128, 16), f32)
    din("w2s", (L, 128, 16, 128), bf16); din("b2_b", (L, D, D), f32)
    for nm in ("ln1g_b", "ln1b_b", "ln2g_b", "ln2b_b"):
        din(nm, (L, D, D), f32)

    y_out = nc.declare_dram_parameter("y_out", [512, D], f32, isOutput=True).ap()
    dbg = {}
    if DEBUG:
        for nm, shape in (("dbg_hT", (D, N)), ("dbg_hhT", (D, N)),
                          ("dbg_aT", (2, 2 * N)), ("dbg_y", (512, D)),
                          ("dbg_y1", (512, D))):
            dbg[nm] = nc.declare_dram_parameter(nm, list(shape), f32,
                                                isOutput=True).ap()

    tableT = nc.dram_tensor("tableT", [N, TW], bf16).ap()
    ag_in = [nc.dram_tensor(f"ag_in{i}", [D, 512], bf16).ap() for i in range(2)]
    ag_out = [nc.dram_tensor(f"ag_out{i}", [NCORES, D, 512], bf16,
                             addr_space="Shared").ap() for i in range(2)]

    with tile.TileContext(nc) as tc:
        from contextlib import ExitStack
        stack = ExitStack()
        consts = stack.enter_context(tc.tile_pool(name="consts", bufs=1))
        persist = stack.enter_context(tc.tile_pool(name="persist", bufs=1))

        def load_const(name, shape, dt, src_ap):
            t = consts.tile(list(shape), dt, tag=name)
            nc.sync.dma_start(out=t[:], in_=src_ap)
            return t

        iota_b = load_const("iota_b", (128, 128), bf16, P["iota_b"][:, :])
        id_f = load_const("id_f32", (128, 128), f32, P["id_f32"][:, :])
        id_b = load_const("id_bf", (128, 128), bf16, P["id_bf"][:, :])
        gbias = load_const("gbias_b", (D, D), f32, P["gbias_b"][:, :])
        eps_t = consts.tile([128, 1], f32, tag="eps")
        nc.vector.memset(eps_t[:], EPS)

        yT_full = persist.tile([D, N], bf16, tag="yT_full")
        y_nat = persist.tile([128, TPC, D], f32, tag="y_nat")
        y1_nat = persist.tile([128, TPC, D], f32, tag="y1_nat")
        y2_nat = persist.tile([128, TPC, D], f32, tag="y2_nat")
        yT_loc = persist.tile([D, 512], bf16, tag="yT_loc")

        # ---------------- stage 1: encoder + tables (replicated) ----------
        with (
            tc.tile_pool(name="enc", bufs=1) as enc,
            tc.tile_pool(name="encw", bufs=2) as encw,
            tc.tile_pool(name="pse", bufs=2, space="PSUM") as pse,
        ):
            xT0 = enc.tile([128, N], f32, tag="xT0")
            xT1 = enc.tile([128, N], f32, tag="xT1")
            nc.sync.dma_start(out=xT0[:], in_=P["xT"][0:128, :])
            nc.sync.dma_start(out=xT1[:], in_=P["xT"][128:256, :])
            ew1a = consts.tile([128, D], f32, tag="ew1a")
            ew1b = consts.tile([128, D], f32, tag="ew1b")
            nc.sync.dma_start(out=ew1a[:], in_=P["ew1"][0:128, :])
            nc.sync.dma_start(out=ew1b[:], in_=P["ew1"][128:256, :])
            ew2 = load_const("ew2", (D, D), bf16, P["ew2"][:, :])
            eb1 = load_const("eb1", (D, 1), f32, P["eb1"][:, :])
            eb2 = load_const("eb2", (D, 1), f32, P["eb2"][:, :])
            gatw = load_const("gatw", (D, D), bf16, P["gatw"][:, :])
            asrcW = load_const("asrcW", (D, 2), bf16, P["asrcW"][:, :])
            adstW = load_const("adstW", (D, 2), bf16, P["adstW"][:, :])

            h1T = enc.tile([D, N], bf16, tag="h1T")
            for nb in range(8):
                sl = slice(512 * nb, 512 * (nb + 1))
                ps = pse.tile([128, 512], f32, tag="pse")
                nc.tensor.matmul(out=ps[:], lhsT=ew1a[:], rhs=xT0[:, sl],
                                 start=True, stop=False)
                nc.tensor.matmul(out=ps[:], lhsT=ew1b[:], rhs=xT1[:, sl],
                                 start=False, stop=True)
                nc.scalar.activation(out=h1T[:, sl], in_=ps[:], func=AF.Relu,
                                     bias=eb1[:], scale=1.0)
            hT = enc.tile([D, N], bf16, tag="hT")
            hhT = enc.tile([D, N], bf16, tag="hhT")
            for nb in range(8):
                sl = slice(512 * nb, 512 * (nb + 1))
                ps = pse.tile([128, 512], f32, tag="pse")
                nc.tensor.matmul(out=ps[:], lhsT=ew2[:], rhs=h1T[:, sl],
                                 start=True, stop=True)
                nc.scalar.activation(out=hT[:, sl], in_=ps[:], func=AF.Identity,
                                     bias=eb2[:], scale=1.0)
            for nb in range(8):
                sl = slice(512 * nb, 512 * (nb + 1))
                ps = pse.tile([128, 512], f32, tag="pse")
                nc.tensor.matmul(out=ps[:], lhsT=gatw[:], rhs=hT[:, sl],
                                 start=True, stop=True)
                nc.vector.tensor_copy(out=hhT[:, sl], in_=ps[:])

            asT = enc.tile([2, N], f32, tag="asT")
            adT = enc.tile([2, N], f32, tag="adT")
            for nb in range(8):
                sl = slice(512 * nb, 512 * (nb + 1))
                ps = pse.tile([2, 512], f32, tag="psa")
                nc.tensor.matmul(out=ps[:], lhsT=asrcW[:], rhs=hhT[:, sl],
                                 start=True, stop=True)
                nc.vector.tensor_copy(out=asT[:, sl], in_=ps[:])
                ps2 = pse.tile([2, 512], f32, tag="psa")
                nc.tensor.matmul(out=ps2[:], lhsT=adstW[:], rhs=hhT[:, sl],
                                 start=True, stop=True)
                nc.vector.tensor_copy(out=adT[:, sl], in_=ps2[:])

            if DEBUG:
                dhT = enc.tile([D, N], f32, tag="dhT")
                nc.gpsimd.tensor_copy(out=dhT[:], in_=hT[:])
                nc.sync.dma_start(out=dbg["dbg_hT"][:, :], in_=dhT[:])
                dhhT = enc.tile([D, N], f32, tag="dhhT")
                nc.gpsimd.tensor_copy(out=dhhT[:], in_=hhT[:])
                nc.sync.dma_start(out=dbg["dbg_hhT"][:, :], in_=dhhT[:])
                daT = enc.tile([2, 2 * N], f32, tag="daT")
                nc.gpsimd.tensor_copy(out=daT[:, 0:N], in_=asT[:])
                nc.gpsimd.tensor_copy(out=daT[:, N : 2 * N], in_=adT[:])
                nc.sync.dma_start(out=dbg["dbg_aT"][:, :], in_=daT[:])

            # gather table rows: [hh(n) x128 | asrc(n) x2 | adst(n) x2 | pad]
            for nt in range(32):
                sl = slice(128 * nt, 128 * (nt + 1))
                pst = pse.tile([128, 128], bf16, tag="pst")
                nc.tensor.transpose(out=pst[:], in_=hhT[:, sl], identity=id_b[:])
                row = encw.tile([128, TW], bf16, tag="row")
                nc.vector.tensor_copy(out=row[:, 0:128], in_=pst[:])
                psu = pse.tile([128, 128], f32, tag="psu")
                nc.tensor.transpose(out=psu[:], in_=asT[:, sl],
                                    identity=id_f[0:2, :])
                nc.vector.tensor_copy(out=row[:, 128:130], in_=psu[:, 0:2])
                psu2 = pse.tile([128, 128], f32, tag="psu")
                nc.tensor.transpose(out=psu2[:], in_=adT[:, sl],
                                    identity=id_f[0:2, :])
                nc.vector.tensor_copy(out=row[:, 130:132], in_=psu2[:, 0:2])
                nc.sync.dma_start(out=tableT[sl, :], in_=row[:])

        # ---------------- stage 2: GAT edge aggregation -------------------
        with (
            tc.tile_pool(name="gg", bufs=2) as gg,          # gathered blocks
            tc.tile_pool(name="gix", bufs=2) as gix,
            tc.tile_pool(name="gsm", bufs=3) as gsm,        # small per-block
            tc.tile_pool(name="gtl", bufs=2) as gtl,        # per-tile
            tc.tile_pool(name="psg", bufs=2, space="PSUM") as psg,
            tc.tile_pool(name="psx", bufs=2, space="PSUM") as psx,
        ):
            for t in range(TPC):
                # a_dst for this tile's 128 dst nodes, via mini dma_gather
                dti = gix.tile([128, 8], i16, tag="dti")
                nc.sync.dma_start(out=dti[:], in_=P["dtidx"][t, :, :])
                drow = gtl.tile([128, 1, TW], bf16, tag="drow")
                nc.gpsimd.dma_gather(out_ap=drow[:, :, :],
                                     in_ap=tableT[:, :], idxs_ap=dti[:, :],
                                     num_idxs=128, num_idxs_reg=128,
                                     elem_size=TW)
                # AdstB[h][p, f] = a_dst[tile_node f, head h]
                adstB = gtl.tile([128, H, 128], bf16, tag="adstB")
                for h in range(H):
                    pst = psx.tile([128, 128], bf16, tag="pstgb")
                    nc.tensor.transpose(out=pst[0:1, :],
                                        in_=drow[:, 0, 130 + h : 131 + h],
                                        identity=id_b[:])
                    adrow = gtl.tile([1, 128], bf16, tag="adrow")
                    nc.vector.tensor_copy(out=adrow[:], in_=pst[0:1, :])
                    nc.gpsimd.partition_broadcast(
                        out_ap=adstB[:, h, :], in_ap=adrow[:],
                        channels=128)

                agg = psg.tile([128, 130], f32, tag="agg")
                for call in range(ncalls):
                    git = gix.tile([128, 64], i16, tag="git")
                    nc.sync.dma_start(out=git[:], in_=P["gidx"][t, call, :, :])
                    G = gg.tile([128, 8, TW], bf16, tag="G")
                    nc.gpsimd.dma_gather(out_ap=G[:, :, :],
                                         in_ap=tableT[:, :], idxs_ap=git[:, :],
                                         num_idxs=1024, num_idxs_reg=1024,
                                         elem_size=TW)
                    drel8 = gsm.tile([128, 8], f32, tag="drel8")
                    nc.sync.dma_start(
                        out=drel8[:],
                        in_=P["dstrel"][t, 8 * call : 8 * (call + 1), :]
                            .rearrange("b p -> p b"))
                    ade8 = gsm.tile([128, 8, 2], f32, tag="ade8")
                    STs = []
                    for bb in range(8):
                        ST = gsm.tile([128, 128], bf16, tag=f"ST{bb}",
                                      name=f"ST{bb}")
                        nc.vector.tensor_scalar(out=ST[:], in0=iota_b[:],
                                                scalar1=drel8[:, bb : bb + 1],
                                                scalar2=None, op0=OP.is_equal)
                        STs.append(ST)
                        tmp = gsm.tile([128, 2, 128], bf16, tag="tmpm")
                        for h in range(H):
                            nc.vector.tensor_tensor(out=tmp[:, h, :], in0=ST[:],
                                                    in1=adstB[:, h, :],
                                                    op=OP.mult)
                        nc.vector.reduce_sum(out=ade8[:, bb, :], in_=tmp[:],
                                             axis=mybir.AxisListType.X)
                    # batched: phi = a_src + a_dst ; ee = exp(lrelu(phi))
                    phi8 = gsm.tile([128, 8, 2], f32, tag="phi8")
                    nc.vector.tensor_tensor(out=phi8[:], in0=G[:, :, 128:130],
                                            in1=ade8[:], op=OP.add)
                    lr8 = gsm.tile([128, 8, 2], f32, tag="lr8")
                    nc.vector.tensor_scalar(out=lr8[:], in0=phi8[:],
                                            scalar1=0.2, scalar2=None,
                                            op0=OP.mult)
                    nc.vector.tensor_tensor(out=lr8[:], in0=lr8[:],
                                            in1=phi8[:], op=OP.max)
                    ee8 = gsm.tile([128, 8, 2], f32, tag="ee8")
                    nc.scalar.activation(out=ee8[:], in_=lr8[:], func=AF.Exp)
                    for bb in range(8):
                        blk = call * 8 + bb
                        vals = gsm.tile([128, 130], bf16, tag="vals")
                        for h in range(H):
                            nc.vector.tensor_scalar(
                                out=vals[:, 64 * h : 64 * (h + 1)],
                                in0=G[:, bb, 64 * h : 64 * (h + 1)],
                                scalar1=ee8[:, bb, h : h + 1], scalar2=None,
                                op0=OP.mult)
                        nc.vector.tensor_copy(out=vals[:, 128:130],
                                              in_=ee8[:, bb, :])
                        nc.tensor.matmul(out=agg[:], lhsT=STs[bb][:],
                                         rhs=vals[:], start=(blk == 0),
                                         stop=(blk == maxb - 1))
                # epilogue: y = agg/denom + bias ; yT
                rec = gsm.tile([128, 2], f32, tag="rec")
                nc.vector.reciprocal(out=rec[:], in_=agg[:, 128:130])
                yt = gtl.tile([128, D], f32, tag="yt")
                for h in range(H):
                    nc.vector.tensor_scalar(out=yt[:, 64 * h : 64 * (h + 1)],
                                            in0=agg[:, 64 * h : 64 * (h + 1)],
                                            scalar1=rec[:, h : h + 1],
                                            scalar2=None, op0=OP.mult)
                nc.vector.tensor_tensor(out=y_nat[:, t, :], in0=yt[:],
                                        in1=gbias[:], op=OP.add)
                pyt = psx.tile([128, 128], f32, tag="pstg")
                nc.tensor.transpose(out=pyt[:], in_=y_nat[:, t, :],
                                    identity=id_f[:])
                nc.vector.tensor_copy(out=yT_loc[:, 128 * t : 128 * (t + 1)],
                                      in_=pyt[:])
            if DEBUG:
                for t in range(TPC):
                    nc.sync.dma_start(out=dbg["dbg_y"][128 * t : 128 * (t + 1), :],
                                      in_=y_nat[:, t, :])

        # ---------------- transformer ------------------------------------
        def allgather(idx):
            nc.sync.dma_start(out=ag_in[idx][:, :], in_=yT_loc[:])
            nc.gpsimd.collective_compute(
                "AllGather", OP.bypass,
                ins=[ag_in[idx]], outs=[ag_out[idx]],
                replica_groups=[list(range(NCORES))])
            nc.sync.dma_start(
                out=yT_full[:].rearrange("d (c n) -> d c n", c=NCORES),
                in_=ag_out[idx].rearrange("c d n -> d c n"))

        allgather(0)

        def cload(nm, shape, dt, src):
            t = consts.tile(list(shape), dt, tag=nm, name=nm)
            nc.sync.dma_start(out=t[:], in_=src)
            return t

        wq_s, wk_s, wv_s, bq_s, bk_s = [], [], [], [], []
        wo_h = [[None] * L for _ in range(NH)]
        w1_s, w2_s, b1c_s = [], [], []
        bcasts = {nm: [] for nm in ("bv_b", "bo_b", "b2_b", "ln1g_b",
                                    "ln1b_b", "ln2g_b", "ln2b_b")}
        for l in range(L):
            wq_s.append(cload(f"wq{l}", (D, D), bf16, P["wq"][l, :, :]))
            wk_s.append(cload(f"wk{l}", (D, D), bf16, P["wk"][l, :, :]))
            wv_s.append(cload(f"wv{l}", (D, D), bf16, P["wv"][l, :, :]))
            for h in range(NH):
                wo_h[h][l] = cload(f"wo{l}_{h}", (DH, D), bf16,
                                   P["wo"][l, DH * h : DH * (h + 1), :])
            bq_s.append(cload(f"bq{l}", (D, 1), f32, P["bq"][l, :, :]))
            bk_s.append(cload(f"bk{l}", (D, 1), f32, P["bk"][l, :, :]))
            for nm in bcasts:
                bcasts[nm].append(cload(f"{nm}{l}", (D, D), f32,
                                        P[nm][l, :, :]))
            w1_s.append(cload(f"w1{l}", (D, FF), bf16, P["w1"][l, :, :]))
            w2_s.append(cload(f"w2{l}", (128, 16, 128), bf16,
                              P["w2s"][l, :, :, :]))
            b1c_s.append(cload(f"b1c{l}", (128, 16), f32, P["b1c"][l, :, :]))

        def layer_norm(dst_ap, src_sb, g_b, b_b, lnp):
            st = lnp.tile([128, 6], f32, tag="lnst")
            nc.vector.bn_stats(out=st[:], in_=src_sb)
            mv = lnp.tile([128, 2], f32, tag="lnmv")
            nc.vector.bn_aggr(out=mv[:], in_=st[:])
            sd = lnp.tile([128, 1], f32, tag="lnsd")
            nc.scalar.activation(out=sd[:], in_=mv[:, 1:2], func=AF.Sqrt,
                                 bias=eps_t[:], scale=1.0)
            nc.vector.reciprocal(out=sd[:], in_=sd[:])
            xc = lnp.tile([128, D], f32, tag="lnxc")
            nc.vector.tensor_scalar(out=xc[:], in0=src_sb,
                                    scalar1=mv[:, 0:1], scalar2=sd[:],
                                    op0=OP.subtract, op1=OP.mult)
            nc.vector.tensor_tensor(out=xc[:], in0=xc[:], in1=g_b, op=OP.mult)
            nc.vector.tensor_tensor(out=dst_ap, in0=xc[:], in1=b_b, op=OP.add)

        for l in range(L):
            yin_nat = y_nat if l == 0 else y2_nat
            yin_T = yT_loc
            with (
                tc.tile_pool(name="tf", bufs=2) as tf,
                tc.tile_pool(name="lnp", bufs=2) as lnp,
                tc.tile_pool(name="es", bufs=3) as es,
                tc.tile_pool(name="psS", bufs=2, space="PSUM") as psS,
                tc.tile_pool(name="psV", bufs=2, space="PSUM") as psV,
                tc.tile_pool(name="psM", bufs=2, space="PSUM") as psM,
            ):
                KT_a = tf.tile([64, N], bf16, tag="KT_a")
                KT_b = tf.tile([64, N], bf16, tag="KT_b")
                for nb in range(8):
                    sl = slice(512 * nb, 512 * (nb + 1))
                    ps = psM.tile([128, 512], f32, tag="psm")
                    nc.tensor.matmul(out=ps[:], lhsT=wk_s[l][:], rhs=yT_full[:, sl],
                                     start=True, stop=True)
                    nc.vector.tensor_scalar(out=KT_a[:, sl], in0=ps[0:64, :],
                                            scalar1=bk_s[l][0:64, :],
                                            scalar2=None, op0=OP.add)
                    nc.vector.tensor_scalar(out=KT_b[:, sl], in0=ps[64:128, :],
                                            scalar1=bk_s[l][64:128, :],
                                            scalar2=None, op0=OP.add)
                vaug = tf.tile([128, 32, NH * 33], bf16, tag="vaug")
                nc.vector.memset(
                    vaug[:].rearrange("p n (h t) -> p n h t", t=33)[:, :, :, 32:33],
                    1.0)
                for nb in range(32):
                    sl = slice(128 * nb, 128 * (nb + 1))
                    ps = psM.tile([128, 512], f32, tag="psm")
                    nc.tensor.matmul(out=ps[:, 0:128], lhsT=yT_full[:, sl],
                                     rhs=wv_s[l][:], start=True, stop=True)
                    nc.vector.tensor_tensor(
                        out=vaug[:, nb, :].rearrange("p (h t) -> p h t",
                                                     t=33)[:, :, 0:32],
                        in0=ps[:, 0:128].rearrange("p (h c) -> p h c", c=32),
                        in1=bcasts["bv_b"][l][:].rearrange("p (h c) -> p h c",
                                                        c=32),
                        op=OP.add)
                QT_a = tf.tile([64, 512], bf16, tag="QT_a")
                QT_b = tf.tile([64, 512], bf16, tag="QT_b")
                ps = psM.tile([128, 512], f32, tag="psm")
                nc.tensor.matmul(out=ps[:], lhsT=wq_s[l][:], rhs=yin_T[:],
                                 start=True, stop=True)
                nc.scalar.activation(out=QT_a[:], in_=ps[0:64, :],
                                     func=AF.Identity, bias=bq_s[l][0:64, :],
                                     scale=1.0)
                nc.scalar.activation(out=QT_b[:], in_=ps[64:128, :],
                                     func=AF.Identity, bias=bq_s[l][64:128, :],
                                     scale=1.0)

                pv01 = psV.tile([97, 512], f32, tag="pv")
                pv23 = psV.tile([97, 512], f32, tag="pv")
                for h in range(NH):
                    pv = pv01 if h < 2 else pv23
                    prow = 64 * (h % 2)
                    for j in range(16):
                        sc = psS.tile([128, 1024], f32, tag="sc")
                        KTh = KT_a if h < 2 else KT_b
                        QTh = QT_a if h < 2 else QT_b
                        hr = DH * (h % 2)
                        for u in range(2):
                            kc = 2 * j + u
                            nc.tensor.matmul(
                                out=sc[:, 512 * u : 512 * (u + 1)],
                                lhsT=KTh[hr : hr + DH,
                                         128 * kc : 128 * (kc + 1)],
                                rhs=QTh[hr : hr + DH, :],
                                start=True, stop=True)
                        ex = es.tile([128, 1024], bf16, tag="ex")
                        nc.scalar.activation(out=ex[:], in_=sc[:], func=AF.Exp,
                                             scale=float(ISQ))
                        for u in range(2):
                            kc = 2 * j + u
                            nc.tensor.matmul(
                                out=pv[prow : prow + 33, :],
                                lhsT=vaug[:, kc, 33 * h : 33 * (h + 1)],
                                rhs=ex[:, 512 * u : 512 * (u + 1)],
                                start=(kc == 0), stop=(kc == 31))
                an_h = []
                for h in range(NH):
                    pv = pv01 if h < 2 else pv23
                    prow = 64 * (h % 2)
                    ah = tf.tile([33, 512], f32, tag=f"ah{h}", name=f"ah{h}")
                    nc.vector.tensor_copy(out=ah[:],
                                          in_=pv[prow : prow + 33, :])
                    rc = lnp.tile([1, 512], f32, tag="rc")
                    nc.vector.reciprocal(out=rc[:], in_=ah[32:33, :])
                    rbh = lnp.tile([33, 512], f32, tag=f"rb{h}", name=f"rb{h}")
                    nc.gpsimd.partition_broadcast(out_ap=rbh[:], in_ap=rc[:],
                                                  channels=33)
                    anh = tf.tile([33, 512], bf16, tag=f"an{h}", name=f"an{h}")
                    nc.vector.tensor_tensor(out=anh[:], in0=ah[:],
                                            in1=rbh[:], op=OP.mult)
                    an_h.append(anh)
                psP = psM.tile([128, 512], f32, tag="psm")
                for h in range(NH):
                    nc.tensor.matmul(out=psP[:], lhsT=wo_h[h][l][:],
                                     rhs=an_h[h][0:32, :],
                                     start=(h == 0), stop=(h == NH - 1))
                outT = tf.tile([D, 512], bf16, tag="outT")
                nc.vector.tensor_copy(out=outT[:], in_=psP[:])

                # natural: residual + bias + LN1 ; build y1T
                y1T = tf.tile([D, 512], bf16, tag="y1T")
                for q in range(4):
                    pn = psV.tile([128, 512], bf16, tag="pv", name="pn")
                    nc.tensor.transpose(out=pn[:, 0:128],
                                        in_=outT[:, 128 * q : 128 * (q + 1)],
                                        identity=id_b[:])
                    res = lnp.tile([128, D], f32, tag="res")
                    nc.vector.tensor_tensor(out=res[:], in0=pn[:, 0:128],
                                            in1=bcasts["bo_b"][l][:], op=OP.add)
                    nc.vector.tensor_tensor(out=res[:], in0=res[:],
                                            in1=yin_nat[:, q, :], op=OP.add)
                    layer_norm(y1_nat[:, q, :], res[:], bcasts["ln1g_b"][l][:],
                               bcasts["ln1b_b"][l][:], lnp)
                    pt = psM.tile([128, 512], f32, tag="psm")
                    nc.tensor.transpose(out=pt[:, 0:128], in_=y1_nat[:, q, :],
                                        identity=id_f[:])
                    nc.vector.tensor_copy(out=y1T[:, 128 * q : 128 * (q + 1)],
                                          in_=pt[:, 0:128])

                # FFN
                f1T = tf.tile([128, 16, 512], bf16, tag="f1T")
                for fc in range(16):
                    ps = psM.tile([128, 512], f32, tag="psm")
                    nc.tensor.matmul(out=ps[:],
                                     lhsT=w1_s[l][:, 128 * fc : 128 * (fc + 1)],
                                     rhs=y1T[:], start=True, stop=True)
                    nc.vector.tensor_scalar(out=f1T[:, fc, :], in0=ps[:],
                                            scalar1=b1c_s[l][:, fc : fc + 1],
                                            scalar2=0.0, op0=OP.add, op1=OP.max)
                psG = psM.tile([128, 512], f32, tag="psm")
                for fc in range(16):
                    nc.tensor.matmul(out=psG[:], lhsT=w2_s[l][:, fc, :],
                                     rhs=f1T[:, fc, :], start=(fc == 0),
                                     stop=(fc == 15))
                f2T = tf.tile([D, 512], bf16, tag="f2T")
                nc.vector.tensor_copy(out=f2T[:], in_=psG[:])
                for q in range(4):
                    pn = psV.tile([128, 512], bf16, tag="pv", name="pn")
                    nc.tensor.transpose(out=pn[:, 0:128],
                                        in_=f2T[:, 128 * q : 128 * (q + 1)],
                                        identity=id_b[:])
                    res = lnp.tile([128, D], f32, tag="res")
                    nc.vector.tensor_tensor(out=res[:], in0=pn[:, 0:128],
                                            in1=bcasts["b2_b"][l][:], op=OP.add)
                    nc.vector.tensor_tensor(out=res[:], in0=res[:],
                                            in1=y1_nat[:, q, :], op=OP.add)
                    layer_norm(y2_nat[:, q, :], res[:], bcasts["ln2g_b"][l][:],
                               bcasts["ln2b_b"][l][:], lnp)
                    pt = psM.tile([128, 512], f32, tag="psm")
                    nc.tensor.transpose(out=pt[:, 0:128], in_=y2_nat[:, q, :],
                                        identity=id_f[:])
                    nc.vector.tensor_copy(out=yT_loc[:, 128 * q : 128 * (q + 1)],
                                          in_=pt[:, 0:128])
            if l == 0:
                allgather(1)
                if DEBUG:
                    for t in range(TPC):
                        nc.sync.dma_start(
                            out=dbg["dbg_y1"][128 * t : 128 * (t + 1), :],
                            in_=y2_nat[:, t, :])

        for q in range(4):
            nc.sync.dma_start(out=y_out[128 * q : 128 * (q + 1), :],
                              in_=y2_nat[:, q, :])
        stack.close()
    nc.compile()
    return nc


_CACHE = {}


def kernel(**inputs) -> np.ndarray:
    in_maps, maxb, ncalls = _host_prep(inputs)
    key = (maxb, ncalls)
    if key not in _CACHE:
        _CACHE[key] = _build(maxb, ncalls)
    nc = _CACHE[key]
    res = run_bass_kernel_spmd(nc, in_maps, core_ids=list(range(NCORES)))
    out = np.concatenate([res.results[c]["y_out"] for c in range(NCORES)], 0)
    if DEBUG:
        kernel.last_results = res
    return out.astype(np.float32)
